# revision 55
# baseline (speedup 1.0000x reference)
"""Trainium2 Bass kernel for nn_Alignment_vector (sparse_attention).

Reference computation per batch b (B=128, Lq=128, Ls=256, d=1024, K=256):
  q = query * matrix                                  (Lq, d)
  A = context @ q.T                                   (Ls, Lq)
  A = leaky_relu(A, 0.1); A = A / ||A||_rows(q-axis)
  attn = softmax(smooth * A.T, axis=s)                (Lq, Ls)
  wc = attn @ context; wc = wc / ||wc||_rows(d-axis)  (Lq, d)
  sim = (query - wc)^2 @ W.T + b; out = sim / ||sim||_rows

Design notes (v3, from the 142us v2):
  - All activation funcs ({Exp, Ln, Square, Copy, Prelu}) live in ONE act
    table set (natural_log_exp_and_others) -> a single ACT_TABLE_LOAD (the
    act-table pass is steered via a scoped get_activation_tables patch in
    _build; default first-match placement thrashed 79 loads = 101us).
    sqrt/rsqrt are computed as exp(+-0.5*ln(x)); n2f must stay < 2^64 for
    the Ln table, hence the SIG down-scale in s_wsq/s_usub.
  - Softmax denominator and the wc/sim norm reciprocals cancel against the
    row l2norms downstream, so we never divide: tu = SIG*wcT - qT*SIG*||wc||
    and the bias is scaled by ||wc||^2 via a K=1 matmul (exact for any b).
  - Context ships in fp8 (both the [d,q]-transposed and natural copies):
    16MB input DMA per core. All fp8 quantization noise washes out through
    the l2norms/softmax (rel err ~7e-3).
  - v3 changes vs v2 (each attacks the DVE(94us)/ACT(82us)/PE(90us) busy
    split measured on the v2 trace):
    * leaky_relu is ONE ACT Prelu(alpha=0.1) instead of Copy+DVE max
      (parametric_relu is resident in every act table set).
    * tqm is bf16, not fp8: DVE TT gets 2x mode (fp8 out forces 1x); the
      mm1 matmul takes fp8 stationary x bf16 moving (legal, same PE speed).
    * mm2 uses fp8 DoubleRow (both operands fp8): 8 matmuls with K=256
      (both s-halves per instruction) instead of 16.
    * the ||wc|| broadcast (psB) is evacuated to SBUF by ACT Copy so umul
      runs in DVE 2x mode (PSUM operand forced the whole op to 1x).
    * the per-(s-half) trs9 softmax row scale is applied by DVE
      tensor_scalar (fp32 scalar operands don't break 2x mode) so ONE wide
      ACT Exp covers both halves.
    * psW is split into two half-tiles (1 PSUM bank each) so the psW
      recycle loop (mm2_{b+2} waits usub_b) runs at half-batch granularity.
    * emission is software-pipelined: group g+1's front half (qm..exp) is
      emitted before group g's back half (mm2..fout) so the in-order PE
      queue always holds a group of mm1 runway.
    * umul/usub/simsq run in d-halves; mm3 opens with the bias matmul
      (ready early) so via subtile deps the h0 chain feeds mm3 j0-3 after
      half the vector work. simsq h0 is a DVE STT directly behind usub_h0
      in the same queue; h1 overlaps on GPSIMD.
    * the mm3->fsq->fnorm->fout tail is emitted per-batch so each psS
      releases after its own chain, not the whole group's (ACT FIFO).
    NOT done, with measured reasons: group-batched norm chains (Ln/Exp
    over 4 batches) serialize the pipeline and cool the PE HAM clock-gate
    (+30us throttle); fp8 DoubleRow mm3 fails numerically (tu^2 spans
    ~1e-3..330, fp8e4 -> rel err 4.5e-2 > 2e-2 budget, verified in numpy);
    manually-rotated shared PSUM bank tiles corrupt batches sharing a bank
    half (missing W-after-R deps, rel err 3.4e-2) -- pool-slot rotation
    with separate tiles is the only sound layout; borrowing a psA-pool
    slot for psN/psB passes the sim but WEDGES the device
    (NRT_EXEC_UNIT_UNRECOVERABLE); qm on GPSIMD starves mm1 (+12us full,
    +9us even half -- an in-order PE queue stalls inside mm1 at the j4
    boundary); simsq fully on DVE (+9us); [1,N] single-partition
    tensor_mul on GPSIMD returns NaN; fout as ACT Copy w/ scale AP +4.3us;
    GRP=5 +11us (ragged groups vs 2-way PSUM rotation).
  - exp(a*trs9)/64 is cast to fp8 on the fly via a memset bias AP on the
    Exp; the /64 keeps e^9 inside fp8 range and cancels per-row.
  - Batches emitted stage-interleaved in groups of 4 (next group's loads
    first) to keep PE continuously busy (p-state) and DMA saturated.
  - PSUM budget exactly 8 banks (bank-granular per buffer): psA 2x, psW 2x
    (evacuated by ACT Square -> tsq, freed at s_usub), psS 2x (psN|psB|psO
    packed in one [128,512] bank tile).
  - tensor_tensor_reduce (DVE ucode) wedges this runtime - use
    scalar_tensor_tensor (TensorScalarPtr) with accum_out instead; DVE
    instructions may read at most ONE operand from PSUM.
"""

import numpy as np
import ml_dtypes

import concourse.bass as bass
import concourse.bacc as bacc
import concourse.tile as tile
from concourse import mybir
from concourse.bass_utils import run_bass_kernel_spmd

B, LQ, LS, D, KS = 128, 128, 256, 1024, 256
NCORES = 8
BLOC = B // NCORES  # batches per core
DC = D // 128       # d chunks
GRP = 4             # batches per pipeline group
F32 = mybir.dt.float32
BF16 = mybir.dt.bfloat16
AF = mybir.ActivationFunctionType
ALU = mybir.AluOpType
PM = mybir.MatmulPerfMode

MM_BF16 = True  # kept for test.py compat
FP8 = mybir.dt.float8e4
SIG = 2.0 ** -7  # wc down-scale; cancels in final l2norm (see s_wcopy)

_cache = {}


def _build(smooth: float, nb: int = BLOC, grp: int = GRP,
           use_prelu: bool = True, qm_bf16: bool = True, mm2_dr: bool = True,
           umul_sbuf: bool = True, group_norms: bool = False):
    key = (smooth, nb, grp, use_prelu, qm_bf16, mm2_dr, umul_sbuf, group_norms)
    if key in _cache:
        return _cache[key]

    nc = bacc.Bacc("TRN2", debug=False)

    hqm = nc.dram_tensor("hqm", (nb, 128, 2048), BF16, kind="ExternalInput")
    hctx = nc.dram_tensor("hctx", (nb, 128, 4096), FP8, kind="ExternalInput")
    hw = nc.dram_tensor("hw", (128, DC, KS), BF16, kind="ExternalInput")
    hb = nc.dram_tensor("hb", (1, KS), BF16, kind="ExternalInput")
    hout = nc.dram_tensor("hout", (nb, LQ, KS), F32, kind="ExternalOutput")

    inv_sm2 = 1.0 / (smooth * smooth)
    QM_DT = BF16 if qm_bf16 else FP8

    with tile.TileContext(nc) as tc:
        with (
            tc.tile_pool(name="const", bufs=1) as cpool,
            tc.tile_pool(name="inp", bufs=3 * grp) as ipool,
            tc.tile_pool(name="work", bufs=grp) as wpool,
            tc.tile_pool(name="ps_a", bufs=2, space="PSUM") as ps_a,
            tc.tile_pool(name="ps_w", bufs=2, space="PSUM") as ps_w,
            tc.tile_pool(name="ps_s", bufs=4, space="PSUM") as ps_s,
        ):
            tW = cpool.tile([128, DC, KS], BF16)
            nc.sync.dma_start(out=tW, in_=hw[:, :, :])
            tb = cpool.tile([1, KS], BF16)
            nc.sync.dma_start(out=tb, in_=hb[:, :])
            tones = cpool.tile([128, 1], BF16)
            nc.vector.memset(tones, 1.0)
            tones1 = cpool.tile([1, 128], BF16)
            nc.vector.memset(tones1, 1.0)
            # bias = -ln(64): te = exp(a*trs9)/64 fits fp8 (max 127)
            tbe = cpool.tile([128, 1], F32)
            nc.vector.memset(tbe, -4.1588830833596715)

            def s_load(v, bi):
                v["tqmT"] = ipool.tile([128, 2048], BF16, tag="tqmT", name="tqmT")
                v["tctx"] = ipool.tile([128, 4096], FP8, tag="tctx", name="tctx")
                nc.sync.dma_start(out=v["tqmT"], in_=hqm[bi])
                nc.sync.dma_start(out=v["tctx"], in_=hctx[bi])
                v["tqT"] = v["tqmT"][:, 0:1024].rearrange("p (j q) -> p j q", j=DC)
                v["tmT"] = v["tqmT"][:, 1024:2048].rearrange("p (j q) -> p j q", j=DC)
                v["tcT"] = v["tctx"][:, 0:2048].rearrange("p (j s) -> p j s", j=DC)
                v["tcn"] = v["tctx"][:, 2048:4096].rearrange("p (i d) -> p i d", i=2)

            def s_qm(v, bi):
                # q*matrix, transposed layout [d, q]; bf16 out -> DVE 2x
                # mode. Stays WHOLLY on DVE: full qm on GPS starved mm1
                # (+12us), and even a GPS h1-half stalls the in-order PE
                # queue inside mm1 at the j4 boundary (+9us measured).
                v["tqm"] = wpool.tile([128, DC, LQ], QM_DT, tag="tqm", name="tqm")
                nc.vector.tensor_mul(
                    v["tqm"].rearrange("p j q -> p (j q)"),
                    v["tqT"].rearrange("p j q -> p (j q)"),
                    v["tmT"].rearrange("p j q -> p (j q)"),
                )

            def s_mm1(v, bi):
                # A[s, q] = sum_d context[s, d] qm[q, d]
                # fp8 stationary x bf16 moving: same PE speed as bf16
                v["psA"] = ps_a.tile([128, 2, LQ], F32, tag="psA", name="psA")
                for i in range(2):
                    for j in range(DC):
                        nc.tensor.matmul(
                            v["psA"][:, i, :],
                            v["tcT"][:, j, 128 * i : 128 * i + 128],
                            v["tqm"][:, j, :],
                            start=(j == 0),
                            stop=(j == DC - 1),
                        )

            def s_prelu(v, bi):
                # leaky_relu(0.1) in ONE ACT op (parametric_relu table entry).
                # Frees psA afterwards.
                v["tal"] = wpool.tile([128, 2, LQ], BF16, tag="tal", name="tal")
                if use_prelu:
                    nc.scalar.activation(
                        v["tal"].rearrange("p a q -> p (a q)"),
                        v["psA"].rearrange("p a q -> p (a q)"),
                        AF.Prelu,
                        alpha=0.1,
                    )
                else:
                    tal01 = wpool.tile([128, 2, LQ], BF16, tag="tal01")
                    nc.scalar.activation(
                        tal01.rearrange("p a q -> p (a q)"),
                        v["psA"].rearrange("p a q -> p (a q)"),
                        AF.Copy,
                        scale=0.1,
                    )
                    nc.vector.tensor_max(
                        v["tal"].rearrange("p a q -> p (a q)"),
                        v["psA"].rearrange("p a q -> p (a q)"),
                        tal01.rearrange("p a q -> p (a q)"),
                    )

            def s_n2a(v, bi):
                # n2A = sum_q leaky^2 / smooth^2, fused square+reduce on DVE:
                # (tal * inv_sm2) * tal with accum_out (one op per s-tile)
                k = v["gk"]
                gv = v["gv"]
                scrA = wpool.tile([128, 2, LQ], BF16, tag="scrA")
                for i in range(2):
                    nc.vector.scalar_tensor_tensor(
                        scrA[:, i, :],
                        v["tal"][:, i, :],
                        inv_sm2,
                        v["tal"][:, i, :],
                        ALU.mult,
                        ALU.mult,
                        accum_out=gv["tn2A"][:, k, i : i + 1],
                    )

            def g_trs(gv, grp_items):
                # trs9 = smooth / ||leaky_row|| = exp(-0.5 * ln(n2A)),
                # one Ln+Exp over the whole group's [128, grp*2] tile
                tlnA = wpool.tile([128, len(grp_items), 2], F32, tag="g_lnA",
                                  bufs=2, name="tlnA")
                nc.scalar.activation(
                    tlnA.rearrange("p g i -> p (g i)"),
                    gv["tn2A"].rearrange("p g i -> p (g i)"),
                    AF.Ln,
                )
                nc.scalar.activation(
                    gv["trs9"].rearrange("p g i -> p (g i)"),
                    tlnA.rearrange("p g i -> p (g i)"),
                    AF.Exp,
                    scale=-0.5,
                )

            def s_exp(v, bi):
                # te = exp(a * trs9 - ln 64) in fp8; the 1/64 (and fp8
                # context) scales wc per-row, which cancels downstream.
                # The per-(s-half) trs9 row scale is applied by DVE
                # tensor_scalar (fp32 scalar operands don't break 2x mode),
                # so ONE wide ACT Exp covers both halves (ACT op count is
                # the bottleneck; each op pays ~220cyc access + seq).
                k = v["gk"]
                gv = v["gv"]
                tals = wpool.tile([128, 2, LQ], BF16, tag="tals", name="tals")
                for i in range(2):
                    nc.vector.tensor_scalar_mul(
                        tals[:, i, :],
                        v["tal"][:, i, :],
                        gv["trs9"][:, k, i : i + 1],
                    )
                v["te"] = wpool.tile([128, 2, LQ], FP8, tag="te", name="te", bufs=2 * grp)
                nc.scalar.activation(
                    v["te"].rearrange("p a q -> p (a q)"),
                    tals.rearrange("p a q -> p (a q)"),
                    AF.Exp,
                    bias=tbe[:, 0:1],
                )

            HDC = DC // 2

            def s_mm2(v, bi):
                # wcT[d, q] = sum_s context[s, d] e[s, q]
                # psW halves share ONE pool tag (2 banks total instead of
                # 4): the psW recycle tightens to mm2_{b+1} <- usub_b, but
                # that chain (~4.6us) is shorter than the DVE per-batch
                # busy (~5.7us) so it doesn't bind -- and the 2 freed banks
                # go to psS (bufs=4), relaxing the MEASURED binding loop
                # (wones_{b+2} <- fout_b, 6-11us stalls on every second
                # batch) to a 4-batch distance.
                v["psW0"] = ps_w.tile([128, HDC, LQ], F32, tag="psWh", name="psW0")
                v["psW1"] = ps_w.tile([128, HDC, LQ], F32, tag="psWh", name="psW1")
                for j in range(DC):
                    ps = v["psW0"] if j < HDC else v["psW1"]
                    jj = j % HDC
                    if mm2_dr:
                        # fp8 DoubleRow: K=256 (both s-halves) per instruction
                        nc.tensor.matmul(
                            ps[:, jj, :],
                            v["tcn"][:, :, 128 * j : 128 * j + 128],
                            v["te"],
                            start=True,
                            stop=True,
                            perf_mode=PM.DoubleRow,
                        )
                    else:
                        for i in range(2):
                            nc.tensor.matmul(
                                ps[:, jj, :],
                                v["tcn"][:, i, 128 * j : 128 * j + 128],
                                v["te"][:, i, :],
                                start=(i == 0),
                                stop=(i == 1),
                            )

            def s_wsq(v, bi):
                # tsq = (SIG*wc)^2 straight from PSUM (scale inside Square).
                # SIG keeps downstream magnitudes in the act-table Ln range
                # (n2f reaches ~5e19 > 2^64 unscaled); every psO row picks up
                # a consistent SIG^2 which the final l2norm cancels.
                v["tsq"] = wpool.tile([128, DC, LQ], BF16, tag="tsq", name="tsq")
                for h in range(2):
                    nc.scalar.activation(
                        v["tsq"][:, h * HDC : (h + 1) * HDC, :]
                        .rearrange("p j q -> p (j q)"),
                        v["psW" + str(h)].rearrange("p j q -> p (j q)"),
                        AF.Square,
                        scale=SIG,
                    )

            def s_wones(v, bi):
                # n2w[1, q] = sum_d wc^2 via ones-matmul partition reduction
                # psN/psB/psO share one PSUM bank tile [128, 512].
                # NOTE: tried borrowing a psA-pool slot for psN/psB so psS
                # holds only psO (to break the wones_{b+2} <- fout_b tail
                # loop): passes the schedule sim but WEDGES the device
                # (NRT_EXEC_UNIT_UNRECOVERABLE) -- do not re-attempt that
                # layout without a hardware-safe validation path.
                psS = ps_s.tile([128, 512], F32, tag="psS", name="psS")
                v["psN"] = psS[0:1, 0:LQ]
                v["psB"] = psS[:, LQ : 2 * LQ]
                v["psO"] = psS[:, 2 * LQ : 2 * LQ + KS]
                for j in range(DC):
                    nc.tensor.matmul(
                        v["psN"],
                        tones,
                        v["tsq"][:, j, :],
                        start=(j == 0),
                        stop=(j == DC - 1),
                    )

            def s_wnorm(v, bi):
                # tnw = ||wc|| = exp(0.5 * ln(n2w))
                tlnw = wpool.tile([1, LQ], F32, tag="tlnw")
                v["tnw"] = wpool.tile([1, LQ], BF16, tag="tnw", name="tnw")
                nc.scalar.activation(tlnw, v["psN"], AF.Ln)
                nc.scalar.activation(v["tnw"], tlnw, AF.Exp, scale=0.5)

            def s_wnsq(v, bi):
                # NOTE: tried gpsimd here (GPS 30% busy vs DVE 59%) -- a
                # [1, LQ] single-partition tensor_mul on the Pool engine
                # produces NaN on hardware; keep it on DVE.
                v["tnwsq"] = wpool.tile([1, LQ], BF16, tag="tnwsq", name="tnwsq")
                nc.vector.tensor_mul(v["tnwsq"], v["tnw"], v["tnw"])

            def s_bcast(v, bi):
                # broadcast ||wc|| across partitions with K=1 matmul
                nc.tensor.matmul(v["psB"], tones1, v["tnw"], start=True, stop=True)

            def s_bevac(v, bi):
                # evacuate the broadcast to SBUF so umul runs in DVE 2x mode
                v["tnwb"] = wpool.tile([128, LQ], BF16, tag="tnwb", name="tnwb")
                nc.scalar.activation(v["tnwb"], v["psB"], AF.Copy)

            def s_umul_h(v, bi, h):
                # u = qT * ||wc|| (broadcast along d-chunks via stride-0).
                # h=None: full width -- one DVE op and one sem set instead
                # of two; the +250ns on the usub_h0 launch costs less than
                # the extra op/sems on the 68%-busy DVE queue.
                src = v["tnwb"] if umul_sbuf else v["psB"]
                nd = DC if h is None else HDC
                j0 = 0 if h is None else h * HDC
                src_b = bass.AP(
                    tensor=src.tensor,
                    offset=src.offset,
                    ap=[list(src.ap[0]), [0, nd], list(src.ap[1])],
                )
                nc.vector.tensor_mul(
                    v["tum"][:, j0 : j0 + nd, :],
                    v["tqT"][:, j0 : j0 + nd, :],
                    src_b,
                )

            def s_usub_h(v, bi, h):
                # tu = SIG*wcT - qT*SIG*||wc|| = -u; sign cancels in square.
                # Frees psW half h (its last reader).
                nc.vector.scalar_tensor_tensor(
                    v["tu"][:, h * HDC : (h + 1) * HDC, :]
                    .rearrange("p j q -> p (j q)"),
                    v["psW" + str(h)].rearrange("p j q -> p (j q)"),
                    SIG,
                    v["tum"][:, h * HDC : (h + 1) * HDC, :]
                    .rearrange("p j q -> p (j q)"),
                    ALU.mult,
                    ALU.subtract,
                )

            def s_simsq_h(v, bi, h):
                # h0 on DVE (STT all-SBUF bf16 2x, directly behind usub_h0
                # in the SAME queue): mm3 j0-3 launch without the GPS 1.1us
                # latency + two sem hops on the tail recycle loop. h1 stays
                # on GPSIMD (33% busy) and overlaps the h0 chain in
                # parallel -- moving h1 to DVE as well measured 9us WORSE.
                if h == 0:
                    nc.vector.scalar_tensor_tensor(
                        v["tsim"][:, 0:HDC, :].rearrange("p j q -> p (j q)"),
                        v["tu"][:, 0:HDC, :].rearrange("p j q -> p (j q)"),
                        1.0,
                        v["tu"][:, 0:HDC, :].rearrange("p j q -> p (j q)"),
                        ALU.mult,
                        ALU.mult,
                    )
                else:
                    nc.gpsimd.tensor_mul(
                        v["tsim"][:, HDC:DC, :].rearrange("p j q -> p (j q)"),
                        v["tu"][:, HDC:DC, :].rearrange("p j q -> p (j q)"),
                        v["tu"][:, HDC:DC, :].rearrange("p j q -> p (j q)"),
                    )

            def s_mm3(v, bi):
                # out[q, k] = ||wc||^2 * b + sum_d sim[q, d] W[k, d]
                # bias FIRST: tnwsq is ready long before tsim, so the
                # accumulation group opens without waiting on the u-chain
                nc.tensor.matmul(v["psO"], v["tnwsq"], tb, start=True, stop=False)
                for j in range(DC):
                    nc.tensor.matmul(
                        v["psO"],
                        v["tsim"][:, j, :],
                        tW[:, j, :],
                        start=False,
                        stop=(j == DC - 1),
                    )

            def s_fsq(v, bi):
                # NOTE: evacuating psO here with an extra ACT Copy (so psS
                # releases before fnorm/fout) measured +1.8us, and using
                # that to group-batch fnorm measured +3.9us -- the ACT copy
                # plus barrier cost more than the shorter recycle loop buys.
                k = v["gk"]
                gv = v["gv"]
                scrO = wpool.tile([128, KS], BF16, tag="scrO")
                nc.scalar.activation(
                    scrO, v["psO"], AF.Square, accum_out=gv["tn2f"][:, k : k + 1]
                )

            def g_fnorm(gv, grp_items):
                # per-PAIR, not per-group: psS (psO region) has only 2 bufs,
                # so a 4-batch fnorm barrier deadlocks (wones of batch 2
                # needs psS back from fout of batch 0, which would wait on
                # fsq of batch 2).
                n = len(grp_items)
                for p0 in range(0, n, 2):
                    pn = min(2, n - p0)
                    tlnf = wpool.tile([128, pn], F32, tag="g_lnf",
                                      bufs=2, name="tlnf")
                    nc.scalar.activation(
                        tlnf, gv["tn2f"][:, p0 : p0 + pn], AF.Ln
                    )
                    nc.scalar.activation(
                        gv["trf"][:, p0 : p0 + pn], tlnf, AF.Exp, scale=-0.5
                    )

            def s_fout(v, bi):
                # on DVE: tried ACT Copy w/ scale AP (same queue as fnorm)
                # -- measured 4.3us SLOWER; the scale-AP Copy path costs
                # more than the saved sem hop
                k = v["gk"]
                gv = v["gv"]
                tout = wpool.tile([128, KS], F32, tag="tout")
                nc.vector.tensor_scalar_mul(tout, v["psO"], gv["trf"][:, k : k + 1])
                nc.sync.dma_start(out=hout[bi], in_=tout)

            def s_utail(v, bi):
                # per-batch back-to-back, in d-HALVES: the h0 chain
                # (umul->usub->simsq->mm3 j0-3, via subtile deps on
                # tum/tu/tsim) launches after half the DVE/GPS work, cutting
                # the tail recycle loop's serial latency
                v["tum"] = wpool.tile([128, DC, LQ], BF16, tag="tum", name="tum")
                v["tu"] = wpool.tile([128, DC, LQ], BF16, tag="tu", name="tu")
                v["tsim"] = wpool.tile([128, DC, LQ], BF16, tag="tsim", name="tsim")
                # (full-width umul measured 128490 vs split's 127338-129142
                # band -- statistically a wash; the split has more samples)
                for h in range(2):
                    s_umul_h(v, bi, h)
                    s_usub_h(v, bi, h)
                    s_simsq_h(v, bi, h)

            def s_otail(v, bi):
                s_mm3(v, bi)
                s_fsq(v, bi)
                s_fnorm1(v, bi)
                s_fout(v, bi)

            # per-batch norm fallbacks when group_norms is off
            def s_trs1(v, bi):
                k = v["gk"]
                gv = v["gv"]
                tlnA = wpool.tile([128, 2], F32, tag="tlnA")
                nc.scalar.activation(tlnA, gv["tn2A"][:, k, :], AF.Ln)
                nc.scalar.activation(gv["trs9"][:, k, :], tlnA, AF.Exp, scale=-0.5)

            def s_fnorm1(v, bi):
                k = v["gk"]
                gv = v["gv"]
                tlnf = wpool.tile([128, 1], F32, tag="tlnf1")
                nc.scalar.activation(tlnf, gv["tn2f"][:, k : k + 1], AF.Ln)
                nc.scalar.activation(gv["trf"][:, k : k + 1], tlnf, AF.Exp, scale=-0.5)

            # Software-pipelined emission: engines execute their streams IN
            # ORDER, so group g+1's front half (qm..exp: PE work = mm1) is
            # emitted BEFORE group g's back half (mm2..fout). The PE then
            # always has ~a group of mm1 runway queued when a back-half
            # matmul briefly stalls on the ACT/DVE chain -- without this the
            # PE idles >3.4us between groups and the HAM clock-gate drops it
            # to 1.2GHz (65us of throttle on the v4 trace).
            front = [
                s_qm, s_mm1, s_prelu, s_n2a,
                (g_trs if group_norms else s_trs1),
                s_exp,
            ]
            back = [
                s_mm2, s_wsq, s_wones, s_wnorm, s_wnsq, s_bcast, s_bevac,
                s_utail, s_otail,
            ]
            group_stages = {g_trs, g_fnorm}

            def run(stages, grp_items):
                for stage in stages:
                    if stage in group_stages:
                        stage(grp_items[0][0]["gv"], grp_items)
                    else:
                        for v, bi in grp_items:
                            stage(v, bi)

            groups = []
            for b0 in range(0, nb, grp):
                gn = min(grp, nb - b0)
                gv = {}
                gv["tn2A"] = wpool.tile([128, gn, 2], F32, tag="g_n2A",
                                        bufs=2, name="tn2A")
                gv["trs9"] = wpool.tile([128, gn, 2], F32, tag="g_trs9",
                                        bufs=2, name="trs9")
                gv["tn2f"] = wpool.tile([128, gn], F32, tag="g_n2f",
                                        bufs=2, name="tn2f")
                gv["trf"] = wpool.tile([128, gn], F32, tag="g_trf",
                                       bufs=2, name="trf")
                groups.append([({"gv": gv, "gk": k}, b0 + k) for k in range(gn)])
            ng = len(groups)
            # prime: loads for groups 0,1 and front half of group 0
            for gi in range(min(2, ng)):
                for v, bi in groups[gi]:
                    s_load(v, bi)
            run(front, groups[0])
            for gi in range(ng):
                if gi + 2 < ng:
                    for v, bi in groups[gi + 2]:
                        s_load(v, bi)
                if gi + 1 < ng:
                    run(front, groups[gi + 1])
                run(back, groups[gi])

    # The act-table-load pass assigns each activation the FIRST table set
    # containing its function; narrow the tables (set indices preserved) so
    # {Exp, Ln, Square, Copy, Prelu} first-match only in
    # natural_log_exp_and_others -> exactly one ACT_TABLE_LOAD.
    _mine = {AF.Exp, AF.Ln, AF.Square, AF.Copy, AF.Identity, AF.Prelu}
    _orig_tables = bacc.get_activation_tables
    def _narrowed(arch):
        full = _orig_tables(arch)
        return {
            name: (set(fns) if name == "natural_log_exp_and_others"
                   else set(fns) - _mine)
            for name, fns in full.items()
        }
    bacc.get_activation_tables = _narrowed
    try:
        nc.compile()
    finally:
        bacc.get_activation_tables = _orig_tables
    _cache[key] = nc
    return nc


def _prep(query, context, matrix, W, b):
    bf = ml_dtypes.bfloat16
    f8 = mybir.dt.np(FP8)
    # [b, p, j, q] = x[b, q, 128j+p]
    qT = query.reshape(B, LQ, DC, 128).transpose(0, 3, 2, 1).reshape(B, 128, 1024)
    mT = matrix.reshape(B, LQ, DC, 128).transpose(0, 3, 2, 1).reshape(B, 128, 1024)
    hqm = np.ascontiguousarray(
        np.concatenate([qT.astype(bf), mT.astype(bf)], axis=2)
    )
    # [b, p, j, s] = context[b, s, 128j+p]
    cT = context.reshape(B, LS, DC, 128).transpose(0, 3, 2, 1).reshape(B, 128, 2048)
    # [b, p, i, d] = context[b, 128i+p, d]
    cn = context.reshape(B, 2, 128, D).transpose(0, 2, 1, 3).reshape(B, 128, 2048)
    hctx = np.ascontiguousarray(
        np.concatenate([cT.astype(f8), cn.astype(f8)], axis=2)
    )
    # [p, j, k] = W[k, 128j+p]
    hw = np.ascontiguousarray(W.reshape(KS, DC, 128).transpose(2, 1, 0)).astype(bf)
    hb = np.ascontiguousarray(b.reshape(1, KS)).astype(bf)
    return hqm, hctx, hw, hb


def kernel(query, context, matrix, W, b, smooth, _trace=False):
    query = np.asarray(query, dtype=np.float32)
    context = np.asarray(context, dtype=np.float32)
    matrix = np.asarray(matrix, dtype=np.float32)
    W = np.asarray(W, dtype=np.float32)
    b = np.asarray(b, dtype=np.float32)

    nc = _build(float(smooth))
    hqm, hctx, hw, hb = _prep(query, context, matrix, W, b)

    in_maps = []
    for c in range(NCORES):
        sl = slice(c * BLOC, (c + 1) * BLOC)
        in_maps.append({"hqm": hqm[sl], "hctx": hctx[sl], "hw": hw, "hb": hb})

    res = run_bass_kernel_spmd(
        nc, in_maps, core_ids=list(range(NCORES)), trace=_trace
    )
    out = np.concatenate([r["hout"] for r in res.results], axis=0)
    out = np.ascontiguousarray(out.astype(np.float32))
    if _trace:
        return out, res
    return out


# revision 57
# speedup vs baseline: 1.2175x; 1.2175x over previous
"""Trainium2 Bass kernel for nn_Alignment_vector (sparse_attention).

Reference computation per batch b (B=128, Lq=128, Ls=256, d=1024, K=256):
  q = query * matrix                                  (Lq, d)
  A = context @ q.T                                   (Ls, Lq)
  A = leaky_relu(A, 0.1); A = A / ||A||_rows(q-axis)
  attn = softmax(smooth * A.T, axis=s)                (Lq, Ls)
  wc = attn @ context; wc = wc / ||wc||_rows(d-axis)  (Lq, d)
  sim = (query - wc)^2 @ W.T + b; out = sim / ||sim||_rows

Design notes (v3, from the 142us v2):
  - All activation funcs ({Exp, Ln, Square, Copy, Prelu}) live in ONE act
    table set (natural_log_exp_and_others) -> a single ACT_TABLE_LOAD (the
    act-table pass is steered via a scoped get_activation_tables patch in
    _build; default first-match placement thrashed 79 loads = 101us).
    sqrt/rsqrt are computed as exp(+-0.5*ln(x)); n2f must stay < 2^64 for
    the Ln table, hence the SIG down-scale in s_wsq/s_usub.
  - Softmax denominator and the wc/sim norm reciprocals cancel against the
    row l2norms downstream, so we never divide: tu = SIG*wcT - qT*SIG*||wc||
    and the bias is scaled by ||wc||^2 via a K=1 matmul (exact for any b).
  - Context ships in fp8 (both the [d,q]-transposed and natural copies):
    16MB input DMA per core. All fp8 quantization noise washes out through
    the l2norms/softmax (rel err ~7e-3).
  - v3 changes vs v2 (each attacks the DVE(94us)/ACT(82us)/PE(90us) busy
    split measured on the v2 trace):
    * leaky_relu is ONE ACT Prelu(alpha=0.1) instead of Copy+DVE max
      (parametric_relu is resident in every act table set).
    * tqm is bf16, not fp8: DVE TT gets 2x mode (fp8 out forces 1x); the
      mm1 matmul takes fp8 stationary x bf16 moving (legal, same PE speed).
    * mm2 uses fp8 DoubleRow (both operands fp8): 8 matmuls with K=256
      (both s-halves per instruction) instead of 16.
    * the ||wc|| broadcast (psB) is evacuated to SBUF by ACT Copy so umul
      runs in DVE 2x mode (PSUM operand forced the whole op to 1x).
    * the per-(s-half) trs9 softmax row scale is applied by DVE
      tensor_scalar (fp32 scalar operands don't break 2x mode) so ONE wide
      ACT Exp covers both halves.
    * psW is split into two half-tiles (1 PSUM bank each) so the psW
      recycle loop (mm2_{b+2} waits usub_b) runs at half-batch granularity.
    * emission is software-pipelined: group g+1's front half (qm..exp) is
      emitted before group g's back half (mm2..fout) so the in-order PE
      queue always holds a group of mm1 runway.
    * umul/usub/simsq run in d-halves; mm3 opens with the bias matmul
      (ready early) so via subtile deps the h0 chain feeds mm3 j0-3 after
      half the vector work. simsq h0 is a DVE STT directly behind usub_h0
      in the same queue; h1 overlaps on GPSIMD.
    * the mm3->fsq->fnorm->fout tail is emitted per-batch so each psS
      releases after its own chain, not the whole group's (ACT FIFO).
    NOT done, with measured reasons: group-batched norm chains (Ln/Exp
    over 4 batches) serialize the pipeline and cool the PE HAM clock-gate
    (+30us throttle); fp8 DoubleRow mm3 fails numerically (tu^2 spans
    ~1e-3..330, fp8e4 -> rel err 4.5e-2 > 2e-2 budget, verified in numpy);
    manually-rotated shared PSUM bank tiles corrupt batches sharing a bank
    half (missing W-after-R deps, rel err 3.4e-2) -- pool-slot rotation
    with separate tiles is the only sound layout; borrowing a psA-pool
    slot for psN/psB passes the sim but WEDGES the device
    (NRT_EXEC_UNIT_UNRECOVERABLE); qm on GPSIMD starves mm1 (+12us full,
    +9us even half -- an in-order PE queue stalls inside mm1 at the j4
    boundary); simsq fully on DVE (+9us); [1,N] single-partition
    tensor_mul on GPSIMD returns NaN; fout as ACT Copy w/ scale AP +4.3us;
    GRP=5 +11us (ragged groups vs 2-way PSUM rotation).
  - exp(a*trs9)/64 is cast to fp8 on the fly via a memset bias AP on the
    Exp; the /64 keeps e^9 inside fp8 range and cancels per-row.
  - Batches emitted stage-interleaved in groups of 4 (next group's loads
    first) to keep PE continuously busy (p-state) and DMA saturated.
  - PSUM budget exactly 8 banks (bank-granular per buffer): psA 2x, psW 2x
    (evacuated by ACT Square -> tsq, freed at s_usub), psS 2x (psN|psB|psO
    packed in one [128,512] bank tile).
  - tensor_tensor_reduce (DVE ucode) wedges this runtime - use
    scalar_tensor_tensor (TensorScalarPtr) with accum_out instead; DVE
    instructions may read at most ONE operand from PSUM.
"""

import numpy as np
import ml_dtypes

import concourse.bass as bass
import concourse.bacc as bacc
import concourse.tile as tile
from concourse import mybir
from concourse.bass_utils import run_bass_kernel_spmd

B, LQ, LS, D, KS = 128, 128, 256, 1024, 256
NCORES = 8
BLOC = B // NCORES  # batches per core
DC = D // 128       # d chunks
GRP = 4             # batches per pipeline group
F32 = mybir.dt.float32
BF16 = mybir.dt.bfloat16
AF = mybir.ActivationFunctionType
ALU = mybir.AluOpType
PM = mybir.MatmulPerfMode

MM_BF16 = True  # kept for test.py compat
FP8 = mybir.dt.float8e4
SIG = 2.0 ** -7  # wc down-scale; cancels in final l2norm (see s_wcopy)

_cache = {}


def _build(smooth: float, nb: int = BLOC, grp: int = GRP,
           use_prelu: bool = True, qm_bf16: bool = True, mm2_dr: bool = True,
           umul_sbuf: bool = True, group_norms: bool = False):
    key = (smooth, nb, grp, use_prelu, qm_bf16, mm2_dr, umul_sbuf, group_norms)
    if key in _cache:
        return _cache[key]

    nc = bacc.Bacc("TRN2", debug=False)

    hqm = nc.dram_tensor("hqm", (nb, 128, 2048), BF16, kind="ExternalInput")
    hctx = nc.dram_tensor("hctx", (nb, 128, 4096), FP8, kind="ExternalInput")
    hw = nc.dram_tensor("hw", (128, DC, KS), BF16, kind="ExternalInput")
    hb = nc.dram_tensor("hb", (1, KS), BF16, kind="ExternalInput")
    hout = nc.dram_tensor("hout", (nb, LQ, KS), F32, kind="ExternalOutput")

    inv_sm2 = 1.0 / (smooth * smooth)
    QM_DT = BF16 if qm_bf16 else FP8

    with tile.TileContext(nc) as tc:
        with (
            tc.tile_pool(name="const", bufs=1) as cpool,
            tc.tile_pool(name="inp", bufs=3 * grp) as ipool,
            tc.tile_pool(name="work", bufs=grp) as wpool,
            tc.tile_pool(name="ps_a", bufs=2, space="PSUM") as ps_a,
            tc.tile_pool(name="ps_w", bufs=2, space="PSUM") as ps_w,
            tc.tile_pool(name="ps_s", bufs=2, space="PSUM") as ps_s,
        ):
            tW = cpool.tile([128, DC, KS], BF16)
            nc.sync.dma_start(out=tW, in_=hw[:, :, :])
            tb = cpool.tile([1, KS], BF16)
            nc.sync.dma_start(out=tb, in_=hb[:, :])
            tones = cpool.tile([128, 1], BF16)
            nc.vector.memset(tones, 1.0)
            tones1 = cpool.tile([1, 128], BF16)
            nc.vector.memset(tones1, 1.0)
            # bias = -ln(64): te = exp(a*trs9)/64 fits fp8 (max 127)
            tbe = cpool.tile([128, 1], F32)
            nc.vector.memset(tbe, -4.1588830833596715)

            def s_load(v, bi):
                v["tqmT"] = ipool.tile([128, 2048], BF16, tag="tqmT", name="tqmT")
                v["tctx"] = ipool.tile([128, 4096], FP8, tag="tctx", name="tctx")
                nc.sync.dma_start(out=v["tqmT"], in_=hqm[bi])
                nc.sync.dma_start(out=v["tctx"], in_=hctx[bi])
                v["tqT"] = v["tqmT"][:, 0:1024].rearrange("p (j q) -> p j q", j=DC)
                v["tmT"] = v["tqmT"][:, 1024:2048].rearrange("p (j q) -> p j q", j=DC)
                v["tcT"] = v["tctx"][:, 0:2048].rearrange("p (j s) -> p j s", j=DC)
                v["tcn"] = v["tctx"][:, 2048:4096].rearrange("p (i d) -> p i d", i=2)

            def s_qm(v, bi):
                # q*matrix, transposed layout [d, q]; bf16 out -> DVE 2x
                # mode. Stays WHOLLY on DVE: full qm on GPS starved mm1
                # (+12us), and even a GPS h1-half stalls the in-order PE
                # queue inside mm1 at the j4 boundary (+9us measured).
                v["tqm"] = wpool.tile([128, DC, LQ], QM_DT, tag="tqm", name="tqm")
                nc.vector.tensor_mul(
                    v["tqm"].rearrange("p j q -> p (j q)"),
                    v["tqT"].rearrange("p j q -> p (j q)"),
                    v["tmT"].rearrange("p j q -> p (j q)"),
                )

            def s_mm1(v, bi):
                # A[s, q] = sum_d context[s, d] qm[q, d]
                # fp8 stationary x bf16 moving: same PE speed as bf16
                v["psA"] = ps_a.tile([128, 2, LQ], F32, tag="psA", name="psA")
                for i in range(2):
                    for j in range(DC):
                        nc.tensor.matmul(
                            v["psA"][:, i, :],
                            v["tcT"][:, j, 128 * i : 128 * i + 128],
                            v["tqm"][:, j, :],
                            start=(j == 0),
                            stop=(j == DC - 1),
                        )

            def s_prelu(v, bi):
                # leaky_relu(0.1) in ONE ACT op (parametric_relu table entry).
                # Frees psA afterwards.
                v["tal"] = wpool.tile([128, 2, LQ], BF16, tag="tal", name="tal")
                if use_prelu:
                    nc.scalar.activation(
                        v["tal"].rearrange("p a q -> p (a q)"),
                        v["psA"].rearrange("p a q -> p (a q)"),
                        AF.Prelu,
                        alpha=0.1,
                    )
                else:
                    tal01 = wpool.tile([128, 2, LQ], BF16, tag="tal01")
                    nc.scalar.activation(
                        tal01.rearrange("p a q -> p (a q)"),
                        v["psA"].rearrange("p a q -> p (a q)"),
                        AF.Copy,
                        scale=0.1,
                    )
                    nc.vector.tensor_max(
                        v["tal"].rearrange("p a q -> p (a q)"),
                        v["psA"].rearrange("p a q -> p (a q)"),
                        tal01.rearrange("p a q -> p (a q)"),
                    )

            def s_n2a(v, bi):
                # n2A = sum_q leaky^2 / smooth^2, fused square+reduce on DVE:
                # (tal * inv_sm2) * tal with accum_out (one op per s-tile)
                k = v["gk"]
                gv = v["gv"]
                scrA = wpool.tile([128, 2, LQ], BF16, tag="scrA")
                for i in range(2):
                    nc.vector.scalar_tensor_tensor(
                        scrA[:, i, :],
                        v["tal"][:, i, :],
                        inv_sm2,
                        v["tal"][:, i, :],
                        ALU.mult,
                        ALU.mult,
                        accum_out=gv["tn2A"][:, k, i : i + 1],
                    )

            def g_trs(gv, grp_items):
                # trs9 = smooth / ||leaky_row|| = exp(-0.5 * ln(n2A)),
                # one Ln+Exp over the whole group's [128, grp*2] tile
                tlnA = wpool.tile([128, len(grp_items), 2], F32, tag="g_lnA",
                                  bufs=2, name="tlnA")
                nc.scalar.activation(
                    tlnA.rearrange("p g i -> p (g i)"),
                    gv["tn2A"].rearrange("p g i -> p (g i)"),
                    AF.Ln,
                )
                nc.scalar.activation(
                    gv["trs9"].rearrange("p g i -> p (g i)"),
                    tlnA.rearrange("p g i -> p (g i)"),
                    AF.Exp,
                    scale=-0.5,
                )

            def s_exp(v, bi):
                # te = exp(a * trs9 - ln 64) in fp8; the 1/64 (and fp8
                # context) scales wc per-row, which cancels downstream.
                # The per-(s-half) trs9 row scale is applied by DVE
                # tensor_scalar (fp32 scalar operands don't break 2x mode),
                # so ONE wide ACT Exp covers both halves (ACT op count is
                # the bottleneck; each op pays ~220cyc access + seq).
                k = v["gk"]
                gv = v["gv"]
                tals = wpool.tile([128, 2, LQ], BF16, tag="tals", name="tals")
                for i in range(2):
                    nc.vector.tensor_scalar_mul(
                        tals[:, i, :],
                        v["tal"][:, i, :],
                        gv["trs9"][:, k, i : i + 1],
                    )
                v["te"] = wpool.tile([128, 2, LQ], FP8, tag="te", name="te", bufs=2 * grp)
                nc.scalar.activation(
                    v["te"].rearrange("p a q -> p (a q)"),
                    tals.rearrange("p a q -> p (a q)"),
                    AF.Exp,
                    bias=tbe[:, 0:1],
                )

            HDC = DC // 2

            def s_mm2(v, bi):
                # wcT[d, q] = sum_s context[s, d] e[s, q]
                # psW is split into two half-tiles (1 PSUM bank each) so the
                # psW recycle loop (mm2_{b+2} waits usub_b) runs at
                # half-batch granularity instead of whole-batch.
                # NOTE: sharing ONE tag for both halves (2 banks, freeing 2
                # for psS bufs=4) tightens the psW recycle to mm2_{b+1} <-
                # usub_b and measured +28us -- with queue contention that
                # 1-batch loop binds everything. Keep 4 psW banks.
                v["psW0"] = ps_w.tile([128, HDC, LQ], F32, tag="psW0", name="psW0")
                v["psW1"] = ps_w.tile([128, HDC, LQ], F32, tag="psW1", name="psW1")
                for j in range(DC):
                    ps = v["psW0"] if j < HDC else v["psW1"]
                    jj = j % HDC
                    if mm2_dr:
                        # fp8 DoubleRow: K=256 (both s-halves) per instruction
                        nc.tensor.matmul(
                            ps[:, jj, :],
                            v["tcn"][:, :, 128 * j : 128 * j + 128],
                            v["te"],
                            start=True,
                            stop=True,
                            perf_mode=PM.DoubleRow,
                        )
                    else:
                        for i in range(2):
                            nc.tensor.matmul(
                                ps[:, jj, :],
                                v["tcn"][:, i, 128 * j : 128 * j + 128],
                                v["te"][:, i, :],
                                start=(i == 0),
                                stop=(i == 1),
                            )

            def s_wsq(v, bi):
                # tsq = (SIG*wc)^2 straight from PSUM (scale inside Square).
                # SIG keeps downstream magnitudes in the act-table Ln range
                # (n2f reaches ~5e19 > 2^64 unscaled); every psO row picks up
                # a consistent SIG^2 which the final l2norm cancels.
                v["tsq"] = wpool.tile([128, DC, LQ], BF16, tag="tsq", name="tsq")
                for h in range(2):
                    nc.scalar.activation(
                        v["tsq"][:, h * HDC : (h + 1) * HDC, :]
                        .rearrange("p j q -> p (j q)"),
                        v["psW" + str(h)].rearrange("p j q -> p (j q)"),
                        AF.Square,
                        scale=SIG,
                    )

            def s_wones(v, bi):
                # n2w[1, q] = sum_d wc^2 via ones-matmul partition reduction
                # psN/psB/psO share one PSUM bank tile [128, 512].
                # NOTE: tried borrowing a psA-pool slot for psN/psB so psS
                # holds only psO (to break the wones_{b+2} <- fout_b tail
                # loop): passes the schedule sim but WEDGES the device
                # (NRT_EXEC_UNIT_UNRECOVERABLE) -- do not re-attempt that
                # layout without a hardware-safe validation path.
                psS = ps_s.tile([128, 512], F32, tag="psS", name="psS")
                v["psN"] = psS[0:1, 0:LQ]
                v["psB"] = psS[:, LQ : 2 * LQ]
                v["psO"] = psS[:, 2 * LQ : 2 * LQ + KS]
                for j in range(DC):
                    nc.tensor.matmul(
                        v["psN"],
                        tones,
                        v["tsq"][:, j, :],
                        start=(j == 0),
                        stop=(j == DC - 1),
                    )

            def s_wnorm(v, bi):
                # tnw = ||wc|| = exp(0.5 * ln(n2w))
                tlnw = wpool.tile([1, LQ], F32, tag="tlnw")
                v["tnw"] = wpool.tile([1, LQ], BF16, tag="tnw", name="tnw")
                nc.scalar.activation(tlnw, v["psN"], AF.Ln)
                nc.scalar.activation(v["tnw"], tlnw, AF.Exp, scale=0.5)

            def s_wnsq(v, bi):
                # NOTE: tried gpsimd here (GPS 30% busy vs DVE 59%) -- a
                # [1, LQ] single-partition tensor_mul on the Pool engine
                # produces NaN on hardware; keep it on DVE.
                v["tnwsq"] = wpool.tile([1, LQ], BF16, tag="tnwsq", name="tnwsq")
                nc.vector.tensor_mul(v["tnwsq"], v["tnw"], v["tnw"])

            def s_bcast(v, bi):
                # broadcast ||wc|| across partitions with K=1 matmul
                nc.tensor.matmul(v["psB"], tones1, v["tnw"], start=True, stop=True)

            def s_bevac(v, bi):
                # evacuate the broadcast to SBUF so umul runs in DVE 2x mode
                v["tnwb"] = wpool.tile([128, LQ], BF16, tag="tnwb", name="tnwb")
                nc.scalar.activation(v["tnwb"], v["psB"], AF.Copy)

            def s_umul_h(v, bi, h):
                # u = qT * ||wc|| (broadcast along d-chunks via stride-0).
                # h=None: full width -- one DVE op and one sem set instead
                # of two; the +250ns on the usub_h0 launch costs less than
                # the extra op/sems on the 68%-busy DVE queue.
                src = v["tnwb"] if umul_sbuf else v["psB"]
                nd = DC if h is None else HDC
                j0 = 0 if h is None else h * HDC
                src_b = bass.AP(
                    tensor=src.tensor,
                    offset=src.offset,
                    ap=[list(src.ap[0]), [0, nd], list(src.ap[1])],
                )
                nc.vector.tensor_mul(
                    v["tum"][:, j0 : j0 + nd, :],
                    v["tqT"][:, j0 : j0 + nd, :],
                    src_b,
                )

            def s_usub_h(v, bi, h):
                # tu = SIG*wcT - qT*SIG*||wc|| = -u; sign cancels in square.
                # Frees psW half h (its last reader).
                nc.vector.scalar_tensor_tensor(
                    v["tu"][:, h * HDC : (h + 1) * HDC, :]
                    .rearrange("p j q -> p (j q)"),
                    v["psW" + str(h)].rearrange("p j q -> p (j q)"),
                    SIG,
                    v["tum"][:, h * HDC : (h + 1) * HDC, :]
                    .rearrange("p j q -> p (j q)"),
                    ALU.mult,
                    ALU.subtract,
                )

            def s_simsq_h(v, bi, h):
                # h0 on DVE (STT all-SBUF bf16 2x, directly behind usub_h0
                # in the SAME queue): mm3 j0-3 launch without the GPS 1.1us
                # latency + two sem hops on the tail recycle loop. h1 stays
                # on GPSIMD (33% busy) and overlaps the h0 chain in
                # parallel -- moving h1 to DVE as well measured 9us WORSE.
                if h == 0:
                    nc.vector.scalar_tensor_tensor(
                        v["tsim"][:, 0:HDC, :].rearrange("p j q -> p (j q)"),
                        v["tu"][:, 0:HDC, :].rearrange("p j q -> p (j q)"),
                        1.0,
                        v["tu"][:, 0:HDC, :].rearrange("p j q -> p (j q)"),
                        ALU.mult,
                        ALU.mult,
                    )
                else:
                    nc.gpsimd.tensor_mul(
                        v["tsim"][:, HDC:DC, :].rearrange("p j q -> p (j q)"),
                        v["tu"][:, HDC:DC, :].rearrange("p j q -> p (j q)"),
                        v["tu"][:, HDC:DC, :].rearrange("p j q -> p (j q)"),
                    )

            def s_mm3(v, bi):
                # out[q, k] = ||wc||^2 * b + sum_d sim[q, d] W[k, d]
                # bias FIRST: tnwsq is ready long before tsim, so the
                # accumulation group opens without waiting on the u-chain
                nc.tensor.matmul(v["psO"], v["tnwsq"], tb, start=True, stop=False)
                for j in range(DC):
                    nc.tensor.matmul(
                        v["psO"],
                        v["tsim"][:, j, :],
                        tW[:, j, :],
                        start=False,
                        stop=(j == DC - 1),
                    )

            def s_fsq(v, bi):
                # NOTE: evacuating psO here with an extra ACT Copy (so psS
                # releases before fnorm/fout) measured +1.8us, and using
                # that to group-batch fnorm measured +3.9us -- the ACT copy
                # plus barrier cost more than the shorter recycle loop buys.
                k = v["gk"]
                gv = v["gv"]
                scrO = wpool.tile([128, KS], BF16, tag="scrO")
                nc.scalar.activation(
                    scrO, v["psO"], AF.Square, accum_out=gv["tn2f"][:, k : k + 1]
                )

            def g_fnorm(gv, grp_items):
                # per-PAIR, not per-group: psS (psO region) has only 2 bufs,
                # so a 4-batch fnorm barrier deadlocks (wones of batch 2
                # needs psS back from fout of batch 0, which would wait on
                # fsq of batch 2).
                n = len(grp_items)
                for p0 in range(0, n, 2):
                    pn = min(2, n - p0)
                    tlnf = wpool.tile([128, pn], F32, tag="g_lnf",
                                      bufs=2, name="tlnf")
                    nc.scalar.activation(
                        tlnf, gv["tn2f"][:, p0 : p0 + pn], AF.Ln
                    )
                    nc.scalar.activation(
                        gv["trf"][:, p0 : p0 + pn], tlnf, AF.Exp, scale=-0.5
                    )

            def s_fout(v, bi):
                # on DVE: tried ACT Copy w/ scale AP (same queue as fnorm)
                # -- measured 4.3us SLOWER; the scale-AP Copy path costs
                # more than the saved sem hop
                k = v["gk"]
                gv = v["gv"]
                tout = wpool.tile([128, KS], F32, tag="tout")
                nc.vector.tensor_scalar_mul(tout, v["psO"], gv["trf"][:, k : k + 1])
                nc.sync.dma_start(out=hout[bi], in_=tout)

            def s_utail(v, bi):
                # per-batch back-to-back, in d-HALVES: the h0 chain
                # (umul->usub->simsq->mm3 j0-3, via subtile deps on
                # tum/tu/tsim) launches after half the DVE/GPS work, cutting
                # the tail recycle loop's serial latency
                v["tum"] = wpool.tile([128, DC, LQ], BF16, tag="tum", name="tum")
                v["tu"] = wpool.tile([128, DC, LQ], BF16, tag="tu", name="tu")
                v["tsim"] = wpool.tile([128, DC, LQ], BF16, tag="tsim", name="tsim")
                # (full-width umul measured 128490 vs split's 127338-129142
                # band -- statistically a wash; the split has more samples)
                for h in range(2):
                    s_umul_h(v, bi, h)
                    s_usub_h(v, bi, h)
                    s_simsq_h(v, bi, h)

            def s_otail(v, bi):
                s_mm3(v, bi)
                s_fsq(v, bi)
                s_fnorm1(v, bi)
                s_fout(v, bi)

            # per-batch norm fallbacks when group_norms is off
            def s_trs1(v, bi):
                k = v["gk"]
                gv = v["gv"]
                tlnA = wpool.tile([128, 2], F32, tag="tlnA")
                nc.scalar.activation(tlnA, gv["tn2A"][:, k, :], AF.Ln)
                nc.scalar.activation(gv["trs9"][:, k, :], tlnA, AF.Exp, scale=-0.5)

            def s_fnorm1(v, bi):
                k = v["gk"]
                gv = v["gv"]
                tlnf = wpool.tile([128, 1], F32, tag="tlnf1")
                nc.scalar.activation(tlnf, gv["tn2f"][:, k : k + 1], AF.Ln)
                nc.scalar.activation(gv["trf"][:, k : k + 1], tlnf, AF.Exp, scale=-0.5)

            # Software-pipelined emission: engines execute their streams IN
            # ORDER, so group g+1's front half (qm..exp: PE work = mm1) is
            # emitted BEFORE group g's back half (mm2..fout). The PE then
            # always has ~a group of mm1 runway queued when a back-half
            # matmul briefly stalls on the ACT/DVE chain -- without this the
            # PE idles >3.4us between groups and the HAM clock-gate drops it
            # to 1.2GHz (65us of throttle on the v4 trace).
            front = [
                s_qm, s_mm1, s_prelu, s_n2a,
                (g_trs if group_norms else s_trs1),
                s_exp,
            ]
            back = [
                s_mm2, s_wsq, s_wones, s_wnorm, s_wnsq, s_bcast, s_bevac,
                s_utail, s_otail,
            ]
            group_stages = {g_trs, g_fnorm}

            def run(stages, grp_items):
                for stage in stages:
                    if stage in group_stages:
                        stage(grp_items[0][0]["gv"], grp_items)
                    else:
                        for v, bi in grp_items:
                            stage(v, bi)

            groups = []
            for b0 in range(0, nb, grp):
                gn = min(grp, nb - b0)
                gv = {}
                gv["tn2A"] = wpool.tile([128, gn, 2], F32, tag="g_n2A",
                                        bufs=2, name="tn2A")
                gv["trs9"] = wpool.tile([128, gn, 2], F32, tag="g_trs9",
                                        bufs=2, name="trs9")
                gv["tn2f"] = wpool.tile([128, gn], F32, tag="g_n2f",
                                        bufs=2, name="tn2f")
                gv["trf"] = wpool.tile([128, gn], F32, tag="g_trf",
                                       bufs=2, name="trf")
                groups.append([({"gv": gv, "gk": k}, b0 + k) for k in range(gn)])
            ng = len(groups)
            # prime: loads for groups 0,1 and front half of group 0
            for gi in range(min(2, ng)):
                for v, bi in groups[gi]:
                    s_load(v, bi)
            run(front, groups[0])
            for gi in range(ng):
                if gi + 2 < ng:
                    for v, bi in groups[gi + 2]:
                        s_load(v, bi)
                if gi + 1 < ng:
                    run(front, groups[gi + 1])
                run(back, groups[gi])

    # The act-table-load pass assigns each activation the FIRST table set
    # containing its function; narrow the tables (set indices preserved) so
    # {Exp, Ln, Square, Copy, Prelu} first-match only in
    # natural_log_exp_and_others -> exactly one ACT_TABLE_LOAD.
    _mine = {AF.Exp, AF.Ln, AF.Square, AF.Copy, AF.Identity, AF.Prelu}
    _orig_tables = bacc.get_activation_tables
    def _narrowed(arch):
        full = _orig_tables(arch)
        return {
            name: (set(fns) if name == "natural_log_exp_and_others"
                   else set(fns) - _mine)
            for name, fns in full.items()
        }
    bacc.get_activation_tables = _narrowed
    try:
        nc.compile()
    finally:
        bacc.get_activation_tables = _orig_tables
    _cache[key] = nc
    return nc


def _prep(query, context, matrix, W, b):
    bf = ml_dtypes.bfloat16
    f8 = mybir.dt.np(FP8)
    # [b, p, j, q] = x[b, q, 128j+p]
    qT = query.reshape(B, LQ, DC, 128).transpose(0, 3, 2, 1).reshape(B, 128, 1024)
    mT = matrix.reshape(B, LQ, DC, 128).transpose(0, 3, 2, 1).reshape(B, 128, 1024)
    hqm = np.ascontiguousarray(
        np.concatenate([qT.astype(bf), mT.astype(bf)], axis=2)
    )
    # [b, p, j, s] = context[b, s, 128j+p]
    cT = context.reshape(B, LS, DC, 128).transpose(0, 3, 2, 1).reshape(B, 128, 2048)
    # [b, p, i, d] = context[b, 128i+p, d]
    cn = context.reshape(B, 2, 128, D).transpose(0, 2, 1, 3).reshape(B, 128, 2048)
    hctx = np.ascontiguousarray(
        np.concatenate([cT.astype(f8), cn.astype(f8)], axis=2)
    )
    # [p, j, k] = W[k, 128j+p]
    hw = np.ascontiguousarray(W.reshape(KS, DC, 128).transpose(2, 1, 0)).astype(bf)
    hb = np.ascontiguousarray(b.reshape(1, KS)).astype(bf)
    return hqm, hctx, hw, hb


def kernel(query, context, matrix, W, b, smooth, _trace=False):
    query = np.asarray(query, dtype=np.float32)
    context = np.asarray(context, dtype=np.float32)
    matrix = np.asarray(matrix, dtype=np.float32)
    W = np.asarray(W, dtype=np.float32)
    b = np.asarray(b, dtype=np.float32)

    nc = _build(float(smooth))
    hqm, hctx, hw, hb = _prep(query, context, matrix, W, b)

    in_maps = []
    for c in range(NCORES):
        sl = slice(c * BLOC, (c + 1) * BLOC)
        in_maps.append({"hqm": hqm[sl], "hctx": hctx[sl], "hw": hw, "hb": hb})

    res = run_bass_kernel_spmd(
        nc, in_maps, core_ids=list(range(NCORES)), trace=_trace
    )
    out = np.concatenate([r["hout"] for r in res.results], axis=0)
    out = np.ascontiguousarray(out.astype(np.float32))
    if _trace:
        return out, res
    return out


# revision 58
# speedup vs baseline: 1.2487x; 1.0256x over previous
"""Trainium2 Bass kernel for nn_Alignment_vector (sparse_attention).

Reference computation per batch b (B=128, Lq=128, Ls=256, d=1024, K=256):
  q = query * matrix                                  (Lq, d)
  A = context @ q.T                                   (Ls, Lq)
  A = leaky_relu(A, 0.1); A = A / ||A||_rows(q-axis)
  attn = softmax(smooth * A.T, axis=s)                (Lq, Ls)
  wc = attn @ context; wc = wc / ||wc||_rows(d-axis)  (Lq, d)
  sim = (query - wc)^2 @ W.T + b; out = sim / ||sim||_rows

Design notes (v3, from the 142us v2):
  - All activation funcs ({Exp, Ln, Square, Copy, Prelu}) live in ONE act
    table set (natural_log_exp_and_others) -> a single ACT_TABLE_LOAD (the
    act-table pass is steered via a scoped get_activation_tables patch in
    _build; default first-match placement thrashed 79 loads = 101us).
    sqrt/rsqrt are computed as exp(+-0.5*ln(x)); n2f must stay < 2^64 for
    the Ln table, hence the SIG down-scale in s_wsq/s_usub.
  - Softmax denominator and the wc/sim norm reciprocals cancel against the
    row l2norms downstream, so we never divide: tu = SIG*wcT - qT*SIG*||wc||
    and the bias is scaled by ||wc||^2 via a K=1 matmul (exact for any b).
  - Context ships in fp8 (both the [d,q]-transposed and natural copies):
    16MB input DMA per core. All fp8 quantization noise washes out through
    the l2norms/softmax (rel err ~7e-3).
  - v3 changes vs v2 (each attacks the DVE(94us)/ACT(82us)/PE(90us) busy
    split measured on the v2 trace):
    * leaky_relu is ONE ACT Prelu(alpha=0.1) instead of Copy+DVE max
      (parametric_relu is resident in every act table set).
    * tqm is bf16, not fp8: DVE TT gets 2x mode (fp8 out forces 1x); the
      mm1 matmul takes fp8 stationary x bf16 moving (legal, same PE speed).
    * mm2 uses fp8 DoubleRow (both operands fp8): 8 matmuls with K=256
      (both s-halves per instruction) instead of 16.
    * the ||wc|| broadcast (psB) is evacuated to SBUF by ACT Copy so umul
      runs in DVE 2x mode (PSUM operand forced the whole op to 1x).
    * the per-(s-half) trs9 softmax row scale is applied by DVE
      tensor_scalar (fp32 scalar operands don't break 2x mode) so ONE wide
      ACT Exp covers both halves.
    * psW is split into two half-tiles (1 PSUM bank each) so the psW
      recycle loop (mm2_{b+2} waits usub_b) runs at half-batch granularity.
    * emission is software-pipelined: group g+1's front half (qm..exp) is
      emitted before group g's back half (mm2..fout) so the in-order PE
      queue always holds a group of mm1 runway.
    * umul/usub/simsq run in d-halves; mm3 opens with the bias matmul
      (ready early) so via subtile deps the h0 chain feeds mm3 j0-3 after
      half the vector work. simsq h0 is a DVE STT directly behind usub_h0
      in the same queue; h1 overlaps on GPSIMD.
    * the mm3->fsq->fnorm->fout tail is emitted per-batch so each psS
      releases after its own chain, not the whole group's (ACT FIFO).
    NOT done, with measured reasons: group-batched norm chains (Ln/Exp
    over 4 batches) serialize the pipeline and cool the PE HAM clock-gate
    (+30us throttle); fp8 DoubleRow mm3 fails numerically (tu^2 spans
    ~1e-3..330, fp8e4 -> rel err 4.5e-2 > 2e-2 budget, verified in numpy);
    manually-rotated shared PSUM bank tiles corrupt batches sharing a bank
    half (missing W-after-R deps, rel err 3.4e-2) -- pool-slot rotation
    with separate tiles is the only sound layout; borrowing a psA-pool
    slot for psN/psB passes the sim but WEDGES the device
    (NRT_EXEC_UNIT_UNRECOVERABLE); qm on GPSIMD starves mm1 (+12us full,
    +9us even half -- an in-order PE queue stalls inside mm1 at the j4
    boundary); simsq fully on DVE (+9us); [1,N] single-partition
    tensor_mul on GPSIMD returns NaN; fout as ACT Copy w/ scale AP +4.3us;
    GRP=5 +11us (ragged groups vs 2-way PSUM rotation).
  - exp(a*trs9)/64 is cast to fp8 on the fly via a memset bias AP on the
    Exp; the /64 keeps e^9 inside fp8 range and cancels per-row.
  - Batches emitted stage-interleaved in groups of 4 (next group's loads
    first) to keep PE continuously busy (p-state) and DMA saturated.
  - PSUM budget exactly 8 banks (bank-granular per buffer): psA 2x, psW 2x
    (evacuated by ACT Square -> tsq, freed at s_usub), psS 2x (psN|psB|psO
    packed in one [128,512] bank tile).
  - tensor_tensor_reduce (DVE ucode) wedges this runtime - use
    scalar_tensor_tensor (TensorScalarPtr) with accum_out instead; DVE
    instructions may read at most ONE operand from PSUM.
"""

import numpy as np
import ml_dtypes

import concourse.bass as bass
import concourse.bacc as bacc
import concourse.tile as tile
from concourse import mybir
from concourse.bass_utils import run_bass_kernel_spmd

B, LQ, LS, D, KS = 128, 128, 256, 1024, 256
NCORES = 8
BLOC = B // NCORES  # batches per core
DC = D // 128       # d chunks
GRP = 4             # batches per pipeline group
F32 = mybir.dt.float32
BF16 = mybir.dt.bfloat16
AF = mybir.ActivationFunctionType
ALU = mybir.AluOpType
PM = mybir.MatmulPerfMode

MM_BF16 = True  # kept for test.py compat
FP8 = mybir.dt.float8e4
SIG = 2.0 ** -7  # wc down-scale; cancels in final l2norm (see s_wcopy)

_cache = {}


def _build(smooth: float, nb: int = BLOC, grp: int = GRP,
           use_prelu: bool = True, qm_bf16: bool = True, mm2_dr: bool = True,
           umul_sbuf: bool = True, group_norms: bool = False):
    key = (smooth, nb, grp, use_prelu, qm_bf16, mm2_dr, umul_sbuf, group_norms)
    if key in _cache:
        return _cache[key]

    nc = bacc.Bacc("TRN2", debug=False)

    hqm = nc.dram_tensor("hqm", (nb, 128, 2048), BF16, kind="ExternalInput")
    hctx = nc.dram_tensor("hctx", (nb, 128, 4096), FP8, kind="ExternalInput")
    hw = nc.dram_tensor("hw", (128, DC, KS), BF16, kind="ExternalInput")
    hb = nc.dram_tensor("hb", (1, KS), BF16, kind="ExternalInput")
    hout = nc.dram_tensor("hout", (nb, LQ, KS), F32, kind="ExternalOutput")

    inv_sm2 = 1.0 / (smooth * smooth)
    QM_DT = BF16 if qm_bf16 else FP8

    with tile.TileContext(nc) as tc:
        with (
            tc.tile_pool(name="const", bufs=1) as cpool,
            tc.tile_pool(name="inp", bufs=3 * grp) as ipool,
            tc.tile_pool(name="work", bufs=grp) as wpool,
            tc.tile_pool(name="ps_a", bufs=1, space="PSUM") as ps_a,
            tc.tile_pool(name="ps_w", bufs=2, space="PSUM") as ps_w,
            tc.tile_pool(name="ps_s", bufs=3, space="PSUM") as ps_s,
        ):
            tW = cpool.tile([128, DC, KS], BF16)
            nc.sync.dma_start(out=tW, in_=hw[:, :, :])
            tb = cpool.tile([1, KS], BF16)
            nc.sync.dma_start(out=tb, in_=hb[:, :])
            tones = cpool.tile([128, 1], BF16)
            nc.vector.memset(tones, 1.0)
            tones1 = cpool.tile([1, 128], BF16)
            nc.vector.memset(tones1, 1.0)
            # bias = -ln(64): te = exp(a*trs9)/64 fits fp8 (max 127)
            tbe = cpool.tile([128, 1], F32)
            nc.vector.memset(tbe, -4.1588830833596715)

            def s_load(v, bi):
                v["tqmT"] = ipool.tile([128, 2048], BF16, tag="tqmT", name="tqmT")
                v["tctx"] = ipool.tile([128, 4096], FP8, tag="tctx", name="tctx")
                nc.sync.dma_start(out=v["tqmT"], in_=hqm[bi])
                nc.sync.dma_start(out=v["tctx"], in_=hctx[bi])
                v["tqT"] = v["tqmT"][:, 0:1024].rearrange("p (j q) -> p j q", j=DC)
                v["tmT"] = v["tqmT"][:, 1024:2048].rearrange("p (j q) -> p j q", j=DC)
                v["tcT"] = v["tctx"][:, 0:2048].rearrange("p (j s) -> p j s", j=DC)
                v["tcn"] = v["tctx"][:, 2048:4096].rearrange("p (i d) -> p i d", i=2)

            def s_qm(v, bi):
                # q*matrix, transposed layout [d, q]; bf16 out -> DVE 2x
                # mode. Stays WHOLLY on DVE: full qm on GPS starved mm1
                # (+12us), and even a GPS h1-half stalls the in-order PE
                # queue inside mm1 at the j4 boundary (+9us measured).
                v["tqm"] = wpool.tile([128, DC, LQ], QM_DT, tag="tqm", name="tqm")
                nc.vector.tensor_mul(
                    v["tqm"].rearrange("p j q -> p (j q)"),
                    v["tqT"].rearrange("p j q -> p (j q)"),
                    v["tmT"].rearrange("p j q -> p (j q)"),
                )

            def s_mm1(v, bi):
                # A[s, q] = sum_d context[s, d] qm[q, d]
                # fp8 stationary x bf16 moving: same PE speed as bf16
                v["psA"] = ps_a.tile([128, 2, LQ], F32, tag="psA", name="psA")
                for i in range(2):
                    for j in range(DC):
                        nc.tensor.matmul(
                            v["psA"][:, i, :],
                            v["tcT"][:, j, 128 * i : 128 * i + 128],
                            v["tqm"][:, j, :],
                            start=(j == 0),
                            stop=(j == DC - 1),
                        )

            def s_prelu(v, bi):
                # leaky_relu(0.1) in ONE ACT op (parametric_relu table entry).
                # Frees psA afterwards.
                v["tal"] = wpool.tile([128, 2, LQ], BF16, tag="tal", name="tal")
                if use_prelu:
                    nc.scalar.activation(
                        v["tal"].rearrange("p a q -> p (a q)"),
                        v["psA"].rearrange("p a q -> p (a q)"),
                        AF.Prelu,
                        alpha=0.1,
                    )
                else:
                    tal01 = wpool.tile([128, 2, LQ], BF16, tag="tal01")
                    nc.scalar.activation(
                        tal01.rearrange("p a q -> p (a q)"),
                        v["psA"].rearrange("p a q -> p (a q)"),
                        AF.Copy,
                        scale=0.1,
                    )
                    nc.vector.tensor_max(
                        v["tal"].rearrange("p a q -> p (a q)"),
                        v["psA"].rearrange("p a q -> p (a q)"),
                        tal01.rearrange("p a q -> p (a q)"),
                    )

            def s_n2a(v, bi):
                # n2A = sum_q leaky^2 / smooth^2, fused square+reduce on DVE:
                # (tal * inv_sm2) * tal with accum_out (one op per s-tile)
                k = v["gk"]
                gv = v["gv"]
                scrA = wpool.tile([128, 2, LQ], BF16, tag="scrA")
                for i in range(2):
                    nc.vector.scalar_tensor_tensor(
                        scrA[:, i, :],
                        v["tal"][:, i, :],
                        inv_sm2,
                        v["tal"][:, i, :],
                        ALU.mult,
                        ALU.mult,
                        accum_out=gv["tn2A"][:, k, i : i + 1],
                    )

            def g_trs(gv, grp_items):
                # trs9 = smooth / ||leaky_row|| = exp(-0.5 * ln(n2A)),
                # one Ln+Exp over the whole group's [128, grp*2] tile
                tlnA = wpool.tile([128, len(grp_items), 2], F32, tag="g_lnA",
                                  bufs=2, name="tlnA")
                nc.scalar.activation(
                    tlnA.rearrange("p g i -> p (g i)"),
                    gv["tn2A"].rearrange("p g i -> p (g i)"),
                    AF.Ln,
                )
                nc.scalar.activation(
                    gv["trs9"].rearrange("p g i -> p (g i)"),
                    tlnA.rearrange("p g i -> p (g i)"),
                    AF.Exp,
                    scale=-0.5,
                )

            def s_exp(v, bi):
                # te = exp(a * trs9 - ln 64) in fp8; the 1/64 (and fp8
                # context) scales wc per-row, which cancels downstream.
                # The per-(s-half) trs9 row scale is applied by DVE
                # tensor_scalar (fp32 scalar operands don't break 2x mode),
                # so ONE wide ACT Exp covers both halves (ACT op count is
                # the bottleneck; each op pays ~220cyc access + seq).
                k = v["gk"]
                gv = v["gv"]
                tals = wpool.tile([128, 2, LQ], BF16, tag="tals", name="tals")
                for i in range(2):
                    nc.vector.tensor_scalar_mul(
                        tals[:, i, :],
                        v["tal"][:, i, :],
                        gv["trs9"][:, k, i : i + 1],
                    )
                v["te"] = wpool.tile([128, 2, LQ], FP8, tag="te", name="te", bufs=2 * grp)
                nc.scalar.activation(
                    v["te"].rearrange("p a q -> p (a q)"),
                    tals.rearrange("p a q -> p (a q)"),
                    AF.Exp,
                    bias=tbe[:, 0:1],
                )

            HDC = DC // 2

            def s_mm2(v, bi):
                # wcT[d, q] = sum_s context[s, d] e[s, q]
                # psW is split into two half-tiles (1 PSUM bank each) so the
                # psW recycle loop (mm2_{b+2} waits usub_b) runs at
                # half-batch granularity instead of whole-batch.
                # NOTE: sharing ONE tag for both halves (2 banks, freeing 2
                # for psS bufs=4) tightens the psW recycle to mm2_{b+1} <-
                # usub_b and measured +28us -- with queue contention that
                # 1-batch loop binds everything. Keep 4 psW banks.
                v["psW0"] = ps_w.tile([128, HDC, LQ], F32, tag="psW0", name="psW0")
                v["psW1"] = ps_w.tile([128, HDC, LQ], F32, tag="psW1", name="psW1")
                for j in range(DC):
                    ps = v["psW0"] if j < HDC else v["psW1"]
                    jj = j % HDC
                    if mm2_dr:
                        # fp8 DoubleRow: K=256 (both s-halves) per instruction
                        nc.tensor.matmul(
                            ps[:, jj, :],
                            v["tcn"][:, :, 128 * j : 128 * j + 128],
                            v["te"],
                            start=True,
                            stop=True,
                            perf_mode=PM.DoubleRow,
                        )
                    else:
                        for i in range(2):
                            nc.tensor.matmul(
                                ps[:, jj, :],
                                v["tcn"][:, i, 128 * j : 128 * j + 128],
                                v["te"][:, i, :],
                                start=(i == 0),
                                stop=(i == 1),
                            )

            def s_wsq(v, bi):
                # tsq = (SIG*wc)^2 straight from PSUM (scale inside Square).
                # SIG keeps downstream magnitudes in the act-table Ln range
                # (n2f reaches ~5e19 > 2^64 unscaled); every psO row picks up
                # a consistent SIG^2 which the final l2norm cancels.
                v["tsq"] = wpool.tile([128, DC, LQ], BF16, tag="tsq", name="tsq")
                for h in range(2):
                    nc.scalar.activation(
                        v["tsq"][:, h * HDC : (h + 1) * HDC, :]
                        .rearrange("p j q -> p (j q)"),
                        v["psW" + str(h)].rearrange("p j q -> p (j q)"),
                        AF.Square,
                        scale=SIG,
                    )

            def s_wones(v, bi):
                # n2w[1, q] = sum_d wc^2 via ones-matmul partition reduction
                # psN/psB/psO share one PSUM bank tile [128, 512].
                # NOTE: tried borrowing a psA-pool slot for psN/psB so psS
                # holds only psO (to break the wones_{b+2} <- fout_b tail
                # loop): passes the schedule sim but WEDGES the device
                # (NRT_EXEC_UNIT_UNRECOVERABLE) -- do not re-attempt that
                # layout without a hardware-safe validation path.
                psS = ps_s.tile([128, 512], F32, tag="psS", name="psS")
                v["psN"] = psS[0:1, 0:LQ]
                v["psB"] = psS[:, LQ : 2 * LQ]
                v["psO"] = psS[:, 2 * LQ : 2 * LQ + KS]
                for j in range(DC):
                    nc.tensor.matmul(
                        v["psN"],
                        tones,
                        v["tsq"][:, j, :],
                        start=(j == 0),
                        stop=(j == DC - 1),
                    )

            def s_wnorm(v, bi):
                # tnw = ||wc|| = exp(0.5 * ln(n2w))
                tlnw = wpool.tile([1, LQ], F32, tag="tlnw")
                v["tnw"] = wpool.tile([1, LQ], BF16, tag="tnw", name="tnw")
                nc.scalar.activation(tlnw, v["psN"], AF.Ln)
                nc.scalar.activation(v["tnw"], tlnw, AF.Exp, scale=0.5)

            def s_wnsq(v, bi):
                # NOTE: tried gpsimd here (GPS 30% busy vs DVE 59%) -- a
                # [1, LQ] single-partition tensor_mul on the Pool engine
                # produces NaN on hardware; keep it on DVE.
                v["tnwsq"] = wpool.tile([1, LQ], BF16, tag="tnwsq", name="tnwsq")
                nc.vector.tensor_mul(v["tnwsq"], v["tnw"], v["tnw"])

            def s_bcast(v, bi):
                # broadcast ||wc|| across partitions with K=1 matmul
                nc.tensor.matmul(v["psB"], tones1, v["tnw"], start=True, stop=True)

            def s_bevac(v, bi):
                # evacuate the broadcast to SBUF so umul runs in DVE 2x mode
                v["tnwb"] = wpool.tile([128, LQ], BF16, tag="tnwb", name="tnwb")
                nc.scalar.activation(v["tnwb"], v["psB"], AF.Copy)

            def s_umul_h(v, bi, h):
                # u = qT * ||wc|| (broadcast along d-chunks via stride-0).
                # h=None: full width -- one DVE op and one sem set instead
                # of two; the +250ns on the usub_h0 launch costs less than
                # the extra op/sems on the 68%-busy DVE queue.
                src = v["tnwb"] if umul_sbuf else v["psB"]
                nd = DC if h is None else HDC
                j0 = 0 if h is None else h * HDC
                src_b = bass.AP(
                    tensor=src.tensor,
                    offset=src.offset,
                    ap=[list(src.ap[0]), [0, nd], list(src.ap[1])],
                )
                nc.vector.tensor_mul(
                    v["tum"][:, j0 : j0 + nd, :],
                    v["tqT"][:, j0 : j0 + nd, :],
                    src_b,
                )

            def s_usub_h(v, bi, h):
                # tu = SIG*wcT - qT*SIG*||wc|| = -u; sign cancels in square.
                # Frees psW half h (its last reader).
                nc.vector.scalar_tensor_tensor(
                    v["tu"][:, h * HDC : (h + 1) * HDC, :]
                    .rearrange("p j q -> p (j q)"),
                    v["psW" + str(h)].rearrange("p j q -> p (j q)"),
                    SIG,
                    v["tum"][:, h * HDC : (h + 1) * HDC, :]
                    .rearrange("p j q -> p (j q)"),
                    ALU.mult,
                    ALU.subtract,
                )

            def s_simsq_h(v, bi, h):
                # h0 on DVE (STT all-SBUF bf16 2x, directly behind usub_h0
                # in the SAME queue): mm3 j0-3 launch without the GPS 1.1us
                # latency + two sem hops on the tail recycle loop. h1 stays
                # on GPSIMD (33% busy) and overlaps the h0 chain in
                # parallel -- moving h1 to DVE as well measured 9us WORSE.
                if h == 0:
                    nc.vector.scalar_tensor_tensor(
                        v["tsim"][:, 0:HDC, :].rearrange("p j q -> p (j q)"),
                        v["tu"][:, 0:HDC, :].rearrange("p j q -> p (j q)"),
                        1.0,
                        v["tu"][:, 0:HDC, :].rearrange("p j q -> p (j q)"),
                        ALU.mult,
                        ALU.mult,
                    )
                else:
                    nc.gpsimd.tensor_mul(
                        v["tsim"][:, HDC:DC, :].rearrange("p j q -> p (j q)"),
                        v["tu"][:, HDC:DC, :].rearrange("p j q -> p (j q)"),
                        v["tu"][:, HDC:DC, :].rearrange("p j q -> p (j q)"),
                    )

            def s_mm3(v, bi):
                # out[q, k] = ||wc||^2 * b + sum_d sim[q, d] W[k, d]
                # bias FIRST: tnwsq is ready long before tsim, so the
                # accumulation group opens without waiting on the u-chain
                nc.tensor.matmul(v["psO"], v["tnwsq"], tb, start=True, stop=False)
                for j in range(DC):
                    nc.tensor.matmul(
                        v["psO"],
                        v["tsim"][:, j, :],
                        tW[:, j, :],
                        start=False,
                        stop=(j == DC - 1),
                    )

            def s_fsq(v, bi):
                # NOTE: evacuating psO here with an extra ACT Copy (so psS
                # releases before fnorm/fout) measured +1.8us, and using
                # that to group-batch fnorm measured +3.9us -- the ACT copy
                # plus barrier cost more than the shorter recycle loop buys.
                k = v["gk"]
                gv = v["gv"]
                scrO = wpool.tile([128, KS], BF16, tag="scrO")
                nc.scalar.activation(
                    scrO, v["psO"], AF.Square, accum_out=gv["tn2f"][:, k : k + 1]
                )

            def g_fnorm(gv, grp_items):
                # per-PAIR, not per-group: psS (psO region) has only 2 bufs,
                # so a 4-batch fnorm barrier deadlocks (wones of batch 2
                # needs psS back from fout of batch 0, which would wait on
                # fsq of batch 2).
                n = len(grp_items)
                for p0 in range(0, n, 2):
                    pn = min(2, n - p0)
                    tlnf = wpool.tile([128, pn], F32, tag="g_lnf",
                                      bufs=2, name="tlnf")
                    nc.scalar.activation(
                        tlnf, gv["tn2f"][:, p0 : p0 + pn], AF.Ln
                    )
                    nc.scalar.activation(
                        gv["trf"][:, p0 : p0 + pn], tlnf, AF.Exp, scale=-0.5
                    )

            def s_fout(v, bi):
                # on DVE: tried ACT Copy w/ scale AP (same queue as fnorm)
                # -- measured 4.3us SLOWER; the scale-AP Copy path costs
                # more than the saved sem hop
                k = v["gk"]
                gv = v["gv"]
                tout = wpool.tile([128, KS], F32, tag="tout")
                nc.vector.tensor_scalar_mul(tout, v["psO"], gv["trf"][:, k : k + 1])
                nc.sync.dma_start(out=hout[bi], in_=tout)

            def s_utail(v, bi):
                # per-batch back-to-back, in d-HALVES: the h0 chain
                # (umul->usub->simsq->mm3 j0-3, via subtile deps on
                # tum/tu/tsim) launches after half the DVE/GPS work, cutting
                # the tail recycle loop's serial latency
                v["tum"] = wpool.tile([128, DC, LQ], BF16, tag="tum", name="tum")
                v["tu"] = wpool.tile([128, DC, LQ], BF16, tag="tu", name="tu")
                v["tsim"] = wpool.tile([128, DC, LQ], BF16, tag="tsim", name="tsim")
                # (full-width umul measured 128490 vs split's 127338-129142
                # band -- statistically a wash; the split has more samples)
                for h in range(2):
                    s_umul_h(v, bi, h)
                    s_usub_h(v, bi, h)
                    s_simsq_h(v, bi, h)

            def s_otail(v, bi):
                s_mm3(v, bi)
                s_fsq(v, bi)
                s_fnorm1(v, bi)
                s_fout(v, bi)

            # per-batch norm fallbacks when group_norms is off
            def s_trs1(v, bi):
                k = v["gk"]
                gv = v["gv"]
                tlnA = wpool.tile([128, 2], F32, tag="tlnA")
                nc.scalar.activation(tlnA, gv["tn2A"][:, k, :], AF.Ln)
                nc.scalar.activation(gv["trs9"][:, k, :], tlnA, AF.Exp, scale=-0.5)

            def s_fnorm1(v, bi):
                k = v["gk"]
                gv = v["gv"]
                tlnf = wpool.tile([128, 1], F32, tag="tlnf1")
                nc.scalar.activation(tlnf, gv["tn2f"][:, k : k + 1], AF.Ln)
                nc.scalar.activation(gv["trf"][:, k : k + 1], tlnf, AF.Exp, scale=-0.5)

            # Software-pipelined emission: engines execute their streams IN
            # ORDER, so group g+1's front half (qm..exp: PE work = mm1) is
            # emitted BEFORE group g's back half (mm2..fout). The PE then
            # always has ~a group of mm1 runway queued when a back-half
            # matmul briefly stalls on the ACT/DVE chain -- without this the
            # PE idles >3.4us between groups and the HAM clock-gate drops it
            # to 1.2GHz (65us of throttle on the v4 trace).
            front = [
                s_qm, s_mm1, s_prelu, s_n2a,
                (g_trs if group_norms else s_trs1),
                s_exp,
            ]
            back = [
                s_mm2, s_wsq, s_wones, s_wnorm, s_wnsq, s_bcast, s_bevac,
                s_utail, s_otail,
            ]
            group_stages = {g_trs, g_fnorm}

            def run(stages, grp_items):
                for stage in stages:
                    if stage in group_stages:
                        stage(grp_items[0][0]["gv"], grp_items)
                    else:
                        for v, bi in grp_items:
                            stage(v, bi)

            groups = []
            for b0 in range(0, nb, grp):
                gn = min(grp, nb - b0)
                gv = {}
                gv["tn2A"] = wpool.tile([128, gn, 2], F32, tag="g_n2A",
                                        bufs=2, name="tn2A")
                gv["trs9"] = wpool.tile([128, gn, 2], F32, tag="g_trs9",
                                        bufs=2, name="trs9")
                gv["tn2f"] = wpool.tile([128, gn], F32, tag="g_n2f",
                                        bufs=2, name="tn2f")
                gv["trf"] = wpool.tile([128, gn], F32, tag="g_trf",
                                       bufs=2, name="trf")
                groups.append([({"gv": gv, "gk": k}, b0 + k) for k in range(gn)])
            ng = len(groups)
            # prime: loads for groups 0,1 and front half of group 0
            for gi in range(min(2, ng)):
                for v, bi in groups[gi]:
                    s_load(v, bi)
            run(front, groups[0])
            for gi in range(ng):
                if gi + 2 < ng:
                    for v, bi in groups[gi + 2]:
                        s_load(v, bi)
                if gi + 1 < ng:
                    run(front, groups[gi + 1])
                run(back, groups[gi])

    # The act-table-load pass assigns each activation the FIRST table set
    # containing its function; narrow the tables (set indices preserved) so
    # {Exp, Ln, Square, Copy, Prelu} first-match only in
    # natural_log_exp_and_others -> exactly one ACT_TABLE_LOAD.
    _mine = {AF.Exp, AF.Ln, AF.Square, AF.Copy, AF.Identity, AF.Prelu}
    _orig_tables = bacc.get_activation_tables
    def _narrowed(arch):
        full = _orig_tables(arch)
        return {
            name: (set(fns) if name == "natural_log_exp_and_others"
                   else set(fns) - _mine)
            for name, fns in full.items()
        }
    bacc.get_activation_tables = _narrowed
    try:
        nc.compile()
    finally:
        bacc.get_activation_tables = _orig_tables
    _cache[key] = nc
    return nc


def _prep(query, context, matrix, W, b):
    bf = ml_dtypes.bfloat16
    f8 = mybir.dt.np(FP8)
    # [b, p, j, q] = x[b, q, 128j+p]
    qT = query.reshape(B, LQ, DC, 128).transpose(0, 3, 2, 1).reshape(B, 128, 1024)
    mT = matrix.reshape(B, LQ, DC, 128).transpose(0, 3, 2, 1).reshape(B, 128, 1024)
    hqm = np.ascontiguousarray(
        np.concatenate([qT.astype(bf), mT.astype(bf)], axis=2)
    )
    # [b, p, j, s] = context[b, s, 128j+p]
    cT = context.reshape(B, LS, DC, 128).transpose(0, 3, 2, 1).reshape(B, 128, 2048)
    # [b, p, i, d] = context[b, 128i+p, d]
    cn = context.reshape(B, 2, 128, D).transpose(0, 2, 1, 3).reshape(B, 128, 2048)
    hctx = np.ascontiguousarray(
        np.concatenate([cT.astype(f8), cn.astype(f8)], axis=2)
    )
    # [p, j, k] = W[k, 128j+p]
    hw = np.ascontiguousarray(W.reshape(KS, DC, 128).transpose(2, 1, 0)).astype(bf)
    hb = np.ascontiguousarray(b.reshape(1, KS)).astype(bf)
    return hqm, hctx, hw, hb


def kernel(query, context, matrix, W, b, smooth, _trace=False):
    query = np.asarray(query, dtype=np.float32)
    context = np.asarray(context, dtype=np.float32)
    matrix = np.asarray(matrix, dtype=np.float32)
    W = np.asarray(W, dtype=np.float32)
    b = np.asarray(b, dtype=np.float32)

    nc = _build(float(smooth))
    hqm, hctx, hw, hb = _prep(query, context, matrix, W, b)

    in_maps = []
    for c in range(NCORES):
        sl = slice(c * BLOC, (c + 1) * BLOC)
        in_maps.append({"hqm": hqm[sl], "hctx": hctx[sl], "hw": hw, "hb": hb})

    res = run_bass_kernel_spmd(
        nc, in_maps, core_ids=list(range(NCORES)), trace=_trace
    )
    out = np.concatenate([r["hout"] for r in res.results], axis=0)
    out = np.ascontiguousarray(out.astype(np.float32))
    if _trace:
        return out, res
    return out
